# revision 33
# baseline (speedup 1.0000x reference)
"""Trainium2 Bass kernel for nn_AllocatingLayer (topk_masking).

Math: out[b,i] = weights[b,i] * [load[b,i] <= 100] where
      load[b,i] = sum_j weights[b,j] * [values[b,j] >= values[b,i]].

Since weights >= 0, load[b,i] is non-increasing in values[b,i], so the mask is
exactly [values[b,i] >= t*_b] for a per-row threshold t*_b, found by searching
F_b(t) = sum_j w[b,j]*[v[b,j] >= t] for the 100-crossing:

- 4 "wide" rounds, 32 probes each (5 bits/round): on a x32-replicated layout
  [128 partitions = 4 rows x 32 probes, 2048 free], each partition evaluates
  the full row at its own probe t = lo + (m+1)*2^(-5k) in ONE fused
  compare-mul-accumulate; the count of probes with F>100 advances lo.
- 3 "fast" evals on the compact layout [128 partitions = 4 rows x 32
  segments, 64 free]: midpoint-state bisection t' = t +- 2^-e, e = 22..24.
- 1 closing eval at t (the bracket around t is (t-2^-24, t+2^-24) and its
  fp32 midpoint IS t): hi = t + [F(t)>100]*2^-24.  The bracket is then <= 1
  ulp wide in the reachable threshold range, so no sample value lies
  strictly inside and the mask [v >= hi] reproduces the reference decision.
  All threshold arithmetic is exact in fp32 (probe offsets are dyadic,
  mantissa spans <= 24 bits).

Sharding: data-parallel over batch, 4 rows per core, no collectives.
Raw bass (no Tile), everything on the Vector engine.  The DVE does not
guarantee a later instruction observes an earlier one's SBUF writes (and the
TensorScalarPtr per-partition scalar is fetched by the sequencer at decode
time), so every instruction is chained through a semaphore — what Tile's
scheduler emits per-op, minus Tile's pre/post overhead.  Cross-partition
row-sum / broadcast use 32x32 stream transpose + free-dim reduce +
stream_shuffle (lane 0 -> all lanes of each 32-block).  v and w are staged
row-major on partitions {0,32,64,96} of ONE combined tile and replicated
x32 by a single stream_shuffle (shuffle cost is free-dim bound).
"""

import os
from contextlib import ExitStack

import numpy as np

import concourse.bacc as bacc
import concourse.bass as bass
import concourse.mybir as mybir
from concourse.bass_utils import run_bass_kernel_spmd

N_CORES = 8
B, K = 32, 2048
RPC = B // N_CORES  # rows per core = 4
SEG = 32  # segments per row
FREE = K // SEG  # 64
P = RPC * SEG  # 128 partitions
N_ROUNDS = 4  # 32-probe rounds, 5 bits each
FAST_EXPS = [22, 23, 24]  # fast-phase +- update exponents
W_RESOURCE = 100.0

_last_exec_ns = None
_last_results = None
_nc_cache = None


def _build_nc():
    nc = bacc.Bacc("TRN2", target_bir_lowering=False)
    f32 = mybir.dt.float32
    AL = mybir.AluOpType
    BCAST0 = [0] * 32  # stream_shuffle mask: every lane <- lane 0 of its block

    v_ext = nc.declare_dram_parameter("values", [RPC, K], f32, isOutput=False)
    w_ext = nc.declare_dram_parameter("weights", [RPC, K], f32, isOutput=False)
    m_ext = nc.declare_dram_parameter("mcol", [P, 1], f32, isOutput=False)
    o_ext = nc.declare_dram_parameter("out", [RPC, K], f32, isOutput=True)

    v_r = v_ext[:].rearrange("r (s f) -> (r s) f", s=SEG)
    w_r = w_ext[:].rearrange("r (s f) -> (r s) f", s=SEG)
    o_r = o_ext[:].rearrange("r (s f) -> (r s) f", s=SEG)

    with ExitStack() as _ctx:
        vwrep = _ctx.enter_context(nc.sbuf_tensor("vwrep", [P, 2 * K], f32))
        vwstage = _ctx.enter_context(nc.sbuf_tensor("vwstage", [P, 2 * K], f32))
        mrep = _ctx.enter_context(nc.sbuf_tensor("mrep", [P, K], f32))
        v128 = _ctx.enter_context(nc.sbuf_tensor("v128", [P, FREE], f32))
        w128 = _ctx.enter_context(nc.sbuf_tensor("w128", [P, FREE], f32))
        mbuf = _ctx.enter_context(nc.sbuf_tensor("mbuf", [P, FREE], f32))
        outt = _ctx.enter_context(nc.sbuf_tensor("outt", [P, FREE], f32))
        mcol = _ctx.enter_context(nc.sbuf_tensor("mcol_sb", [P, 1], f32))
        pad = _ctx.enter_context(nc.sbuf_tensor("pad", [P, SEG], f32))
        tpbuf = _ctx.enter_context(nc.sbuf_tensor("tpbuf", [P, SEG], f32))
        cols = _ctx.enter_context(nc.sbuf_tensor("cols", [P, 8], f32))
        dma_sem = _ctx.enter_context(nc.semaphore("dma_sem"))
        dma_sem2 = _ctx.enter_context(nc.semaphore("dma_sem2"))
        dma_sem3 = _ctx.enter_context(nc.semaphore("dma_sem3"))
        dma_sem4 = _ctx.enter_context(nc.semaphore("dma_sem4"))
        dma_sem5 = _ctx.enter_context(nc.semaphore("dma_sem5"))
        done_sem = _ctx.enter_context(nc.semaphore("done_sem"))
        vsem = _ctx.enter_context(nc.semaphore("vsem"))
        init_sem = _ctx.enter_context(nc.semaphore("init_sem"))
        block = _ctx.enter_context(nc.Block(no_gpsimd_drain=True))

        t_a = cols[:, 0:1]
        t_b = cols[:, 1:2]
        fcol = cols[:, 2:3]
        dnpm = cols[:, 3:4]
        dnb = cols[:, 4:5]
        lo = cols[:, 5:6]
        hi = cols[:, 6:7]

        # v rows -> cols 0:K, w rows -> cols K:2K of the combined staging
        # tile, on partitions {0,32,64,96}; ONE stream_shuffle broadcasts
        # both within each 32-partition block.
        vrep = vwrep[:, 0:K]
        wrep = vwrep[:, K : 2 * K]
        vstage_rows = bass.AP(
            tensor=vwstage, offset=0, ap=[[SEG * 2 * K, RPC], [1, K]]
        )
        wstage_rows = bass.AP(
            tensor=vwstage, offset=K, ap=[[SEG * 2 * K, RPC], [1, K]]
        )

        sim_init = bool(os.environ.get("KERNEL_SIM_INIT"))

        @block.sync
        def _(sync):
            if sim_init:
                sync.wait_ge(init_sem, 1)
            sync.dma_start(out=vstage_rows, in_=v_ext[:]).then_inc(dma_sem, 16)
            sync.dma_start(out=mcol[:], in_=m_ext[:]).then_inc(dma_sem5, 16)
            sync.dma_start(out=v128[:], in_=v_r).then_inc(dma_sem3, 16)
            sync.wait_ge(done_sem, 1)
            sync.dma_start(out=o_r, in_=outt[:]).then_inc(dma_sem, 16)

        @block.scalar
        def _(scalar):
            if sim_init:
                scalar.wait_ge(init_sem, 1)
            scalar.dma_start(out=wstage_rows, in_=w_ext[:]).then_inc(dma_sem2, 16)
            scalar.dma_start(out=w128[:], in_=w_r).then_inc(dma_sem4, 16)

        @block.vector
        def _(vector):
            vcnt = [0]

            def chain(inst, inc=True):
                if vcnt[0]:
                    inst._wait_ge(vsem, vcnt[0])
                if inc:
                    vcnt[0] += 1
                    inst.then_inc(vsem, 1)
                return inst

            if sim_init:
                # CoreSim flags reads of uninitialized SBUF; the shuffle's AP
                # spans all lanes though only {0,32,64,96} are used.  Sim-only
                # init, with the DMAs ordered behind it via init_sem.
                chain(nc.vector.memset(vwstage[:], 0.0), inc=False).then_inc(
                    init_sem, 1
                )
            chain(nc.vector.memset(pad[:], 0.0))
            chain(nc.vector.memset(lo, 0.0))

            vector.wait_ge(dma_sem, 16)  # v rows
            vector.wait_ge(dma_sem2, 16)  # w rows
            chain(nc.vector.stream_shuffle(vwrep[:], vwstage[:], BCAST0))

            def count_bcast():
                """pad col0 -> per-row sum at lane0 -> broadcast into dnb."""
                chain(nc.vector.transpose(tpbuf[:], pad[:]))
                chain(nc.vector.reduce_sum(dnpm, tpbuf[:], axis=mybir.AxisListType.X))
                chain(nc.vector.stream_shuffle(dnb, dnpm, BCAST0))

            # ---- wide rounds: 32 probes, 5 bits each ----
            vector.wait_ge(dma_sem5, 16)  # mcol
            for k in range(N_ROUNDS):
                step = float(2.0 ** (-5 * (k + 1)))
                # t[p] = mcol[p]*step + lo   (mcol = (p%32)+1)
                chain(
                    nc.vector.scalar_tensor_tensor(
                        out=t_a,
                        in0=mcol[:],
                        scalar=step,
                        in1=lo,
                        op0=AL.mult,
                        op1=AL.add,
                    )
                )
                chain(
                    nc.vector.scalar_tensor_tensor(
                        out=mrep[:],
                        in0=vrep,
                        scalar=t_a,
                        in1=wrep,
                        op0=AL.is_ge,
                        op1=AL.mult,
                        accum_out=fcol,
                    )
                )
                # bits = (F > 100) into pad col 0; count + broadcast
                chain(
                    nc.vector.tensor_scalar(
                        out=pad[:, 0:1],
                        in0=fcol,
                        scalar1=W_RESOURCE,
                        scalar2=None,
                        op0=AL.is_gt,
                    )
                )
                count_bcast()  # dnb = count c, broadcast per row
                # lo += c*step
                chain(
                    nc.vector.scalar_tensor_tensor(
                        out=lo,
                        in0=dnb,
                        scalar=step,
                        in1=lo,
                        op0=AL.mult,
                        op1=AL.add,
                    )
                )

            # ---- fast phase on compact layout ----
            def f_eval(thr_col):
                chain(
                    nc.vector.scalar_tensor_tensor(
                        out=mbuf[:],
                        in0=v128[:],
                        scalar=thr_col,
                        in1=w128[:],
                        op0=AL.is_ge,
                        op1=AL.mult,
                        accum_out=pad[:, 0:1],
                    )
                )
                chain(nc.vector.transpose(tpbuf[:], pad[:]))
                chain(nc.vector.reduce_sum(fcol, tpbuf[:], axis=mybir.AxisListType.X))

            vector.wait_ge(dma_sem3, 16)  # v128
            vector.wait_ge(dma_sem4, 16)  # w128
            first_half = float(2.0 ** -(5 * N_ROUNDS + 1))
            chain(
                nc.vector.tensor_scalar(
                    out=t_a, in0=lo, scalar1=first_half, scalar2=None, op0=AL.add
                )
            )
            t_cur, t_nxt = t_a, t_b
            for e in FAST_EXPS:
                f_eval(t_cur)
                chain(
                    nc.vector.tensor_scalar(
                        out=dnpm,
                        in0=fcol,
                        scalar1=W_RESOURCE,
                        scalar2=0.5,
                        op0=AL.is_gt,
                        op1=AL.subtract,
                    )
                )
                chain(nc.vector.stream_shuffle(dnb, dnpm, BCAST0))
                chain(
                    nc.vector.scalar_tensor_tensor(
                        out=t_nxt,
                        in0=dnb,
                        scalar=float(2.0 ** -(e - 1)),
                        in1=t_cur,
                        op0=AL.mult,
                        op1=AL.add,
                    )
                )
                t_cur, t_nxt = t_nxt, t_cur

            # ---- closing eval: hi = t + [F(t)>100]*2^-24 ----
            f_eval(t_cur)
            chain(
                nc.vector.tensor_scalar(
                    out=dnpm,
                    in0=fcol,
                    scalar1=W_RESOURCE,
                    scalar2=None,
                    op0=AL.is_gt,
                )
            )
            chain(nc.vector.stream_shuffle(dnb, dnpm, BCAST0))
            chain(
                nc.vector.scalar_tensor_tensor(
                    out=hi,
                    in0=dnb,
                    scalar=float(2.0 ** -FAST_EXPS[-1]),
                    in1=t_cur,
                    op0=AL.mult,
                    op1=AL.add,
                )
            )

            # ---- fused final mask: out = (v >= hi) * w ----
            chain(
                nc.vector.scalar_tensor_tensor(
                    out=outt[:],
                    in0=v128[:],
                    scalar=hi,
                    in1=w128[:],
                    op0=AL.is_ge,
                    op1=AL.mult,
                ),
                inc=False,
            ).then_inc(done_sem, 1)

    nc.compile()
    return nc


def _mcol():
    return np.ascontiguousarray(
        ((np.arange(P) % SEG) + 1).astype(np.float32).reshape(P, 1)
    )


def kernel(values, weights):
    global _nc_cache, _last_exec_ns, _last_results
    v = np.ascontiguousarray(np.asarray(values, dtype=np.float32))
    w = np.ascontiguousarray(np.asarray(weights, dtype=np.float32))
    assert v.shape == (B, K) and w.shape == (B, K)
    if _nc_cache is None:
        _nc_cache = _build_nc()
    mc = _mcol()
    in_maps = [
        {
            "values": np.ascontiguousarray(v[i * RPC : (i + 1) * RPC]),
            "weights": np.ascontiguousarray(w[i * RPC : (i + 1) * RPC]),
            "mcol": mc,
        }
        for i in range(N_CORES)
    ]
    trace = bool(os.environ.get("KERNEL_TRACE"))
    res = run_bass_kernel_spmd(
        _nc_cache, in_maps, core_ids=list(range(N_CORES)), trace=trace
    )
    _last_exec_ns = res.exec_time_ns
    _last_results = res
    return np.concatenate([res.results[i]["out"] for i in range(N_CORES)], axis=0)
